# revision 2
# baseline (speedup 1.0000x reference)
"""Trainium2 Bass kernel for nn_FFNet_17600775979626.

Spiking FFN layer: cur = einsum('tbi,oi->tbo', x, W) + b, followed by a
leaky-integrate-and-fire scan over T with subtractive reset (snntorch Leaky,
beta=0.95, threshold=1.0). Returns spk_rec [T, B, NO] (0.0/1.0 floats).

Distribution: output-neuron sharding. Each of the 8 cores computes all
(T, B) for a 256-wide slice of the 2048 output neurons. x is replicated
(transposed on host so the contraction dim lands on SBUF partitions); W
planes and b are sliced per core.

GEMM modes:
 hybrid8 (default): fp16 main pass (xh16*Wh16, exact products) plus two
         fp8e4 DoubleRow correction passes accumulated in a second PSUM
         tile at product scale 2^16:
            corr = xl8*W8 + xh8*Wl8
         with xl8 = e4m3((x - fp16(x)) * 2^11), W8 = e4m3(W * 2^5),
         Wl8 = e4m3((W - fp16(W)) * 2^16), and xh8 = e4m3(fp16(x))
         converted on-chip by the Activation engine. DoubleRow contracts
         2x128 K per instruction at 0.5 cycles/row, so each correction
         pass costs 1/4 of an fp16 pass: 6144 PE cycles/step total vs
         12288 for fp16x2. cur = main + corr * 2^-16 is accurate to
         ~2e-5 (measured 250/33.5M spike flips, rel err ~0.009).
 fp16x2: x and W split into fp16 (hi, lo) pairs on host; three 1-cycle/row
         passes xh*Wh + xh*Wl + xl*Wh (fp16 products exact in fp32).

Spikes are produced as uint8 in [B, T, O_S] layout (contiguous 1 KiB DMA
runs) and converted/transposed to fp32 [T, B, NO] on host.

Recurrence (TH=1): m_t = w_{t-1} + cur_t;  spk_t = (m_t > 1);
                   w_t = beta*m_t - spk_t + b  (bias folded into carry).

Walrus codegen on this target accepts at most ONE sync-wait command per
engine instruction, while Tile's wait assigner freely emits several. Two
post-scheduling passes fix that: _slim_waits drops waits already implied
transitively (per-queue FIFO dispatch + semaphore vector clocks), and
_split_waits moves any excess waits onto injected same-queue NoOps.
"""

import os

import numpy as np

T, B, NI, NO = 128, 128, 2048, 2048
NCORES = 8
O_S = NO // NCORES  # 256 output neurons per core
KC = NI // 128  # 16 contraction chunks
BETA = 0.95
CORR_SCALE = float(2.0**-16)

MODE = os.environ.get("KERNEL_MODE", "hybrid8")

_cache = {}


def _build_nc(mode):
    if mode == "hybrid8":
        return _build_nc_hybrid8()
    return _build_nc_fp16(mode)


def _build_nc_hybrid8():
    from contextlib import ExitStack

    import concourse.bass as bass
    import concourse.mybir as mybir
    import concourse.tile as tile

    f32 = mybir.dt.float32
    f16 = mybir.dt.float16
    f8 = mybir.dt.float8e4
    u8 = mybir.dt.uint8
    DR = mybir.MatmulPerfMode.DoubleRow

    nc = bass.Bass()
    xh16 = nc.declare_dram_parameter("xh16", [NI, T * B], f16, isOutput=False)
    xl8 = nc.declare_dram_parameter("xl8", [NI, T * B], f8, isOutput=False)
    Wh16 = nc.declare_dram_parameter("Wh16", [NI, O_S], f16, isOutput=False)
    W8 = nc.declare_dram_parameter("W8", [NI, O_S], f8, isOutput=False)
    Wl8 = nc.declare_dram_parameter("Wl8", [NI, O_S], f8, isOutput=False)
    # cols 0..127: ones (lhsT of K=1 bias seed matmuls); then bias hi, lo
    ob = nc.declare_dram_parameter("ob", [1, 128 + 2 * O_S], f16, isOutput=False)
    # b-major so each (b, tq) store is a contiguous TQ*O_S run
    spk = nc.declare_dram_parameter("spk", [B, T, O_S], u8, isOutput=True)

    TQ = 4  # timesteps per DMA batch
    assert T % TQ == 0

    with tile.TileContext(nc) as tc, ExitStack() as ctx:
        singles = ctx.enter_context(tc.tile_pool(name="singles", bufs=1))
        xpool = ctx.enter_context(tc.tile_pool(name="xp", bufs=2))
        cpool = ctx.enter_context(tc.tile_pool(name="cp", bufs=2))
        spool = ctx.enter_context(tc.tile_pool(name="sp", bufs=3))
        fpool = ctx.enter_context(tc.tile_pool(name="fp", bufs=2))
        psm = ctx.enter_context(tc.tile_pool(name="psm", bufs=3, space="PSUM"))
        psc = ctx.enter_context(tc.tile_pool(name="psc", bufs=3, space="PSUM"))

        # chunk k = 128*kc + p on partitions; DoubleRow pair = kc slices
        # (2c, 2c+1), i.e. k = 256c + 128i + p, taken as [:, 2c:2c+2, ...]
        xh16r = xh16[:].rearrange("(k p) tb -> p k tb", p=128)
        xl8r = xl8[:].rearrange("(k p) tb -> p k tb", p=128)
        Wh16r = Wh16[:].rearrange("(k p) o -> p k o", p=128)
        W8r = W8[:].rearrange("(k p) o -> p k o", p=128)
        Wl8r = Wl8[:].rearrange("(k p) o -> p k o", p=128)

        # Prefetch the first timestep batch before the W preload so the
        # first matmuls start as early as possible.
        xt0 = xpool.tile([128, KC, TQ * B], f16)
        nc.sync.dma_start(out=xt0[:], in_=xh16r[:, :, : TQ * B])
        xl0 = xpool.tile([128, KC, TQ * B], f8)
        nc.sync.dma_start(out=xl0[:], in_=xl8r[:, :, : TQ * B])

        wt16 = singles.tile([128, KC, O_S], f16)
        nc.sync.dma_start(out=wt16[:], in_=Wh16r[:])
        w8sb = singles.tile([128, KC, O_S], f8)
        nc.sync.dma_start(out=w8sb[:], in_=W8r[:])
        wl8sb = singles.tile([128, KC, O_S], f8)
        nc.sync.dma_start(out=wl8sb[:], in_=Wl8r[:])

        ob_sb = singles.tile([1, 128 + 2 * O_S], f16)
        nc.sync.dma_start(out=ob_sb[:], in_=ob[:])

        m_sb = singles.tile([128, O_S], f32)  # membrane potential
        w_sb = singles.tile([128, O_S], f32)  # carry: beta*m + b - spk

        # One-time bias broadcast via ones x bias seed matmuls; bias_full
        # feeds the per-step carry update and the initial carry.
        bias_full = singles.tile([128, O_S], f32)
        ps_b = psm.tile([128, O_S], f32, tag="m")
        for h in range(2):
            nc.tensor.matmul(
                ps_b[:],
                lhsT=ob_sb[:, :128],
                rhs=ob_sb[:, 128 + h * O_S : 128 + (h + 1) * O_S],
                start=(h == 0),
                stop=(h == 1),
            )
        nc.vector.tensor_copy(bias_full[:], ps_b[:])
        nc.vector.tensor_copy(w_sb[:], bias_full[:])  # w_init = b

        for tq in range(T // TQ):
            if tq == 0:
                xt, xl = xt0, xl0
            else:
                xt = xpool.tile([128, KC, TQ * B], f16)
                nc.sync.dma_start(
                    out=xt[:], in_=xh16r[:, :, tq * TQ * B : (tq + 1) * TQ * B]
                )
                xl = xpool.tile([128, KC, TQ * B], f8)
                nc.sync.dma_start(
                    out=xl[:], in_=xl8r[:, :, tq * TQ * B : (tq + 1) * TQ * B]
                )
            # xh8 = e4m3(fp16(x)) converted on the otherwise-idle Act engine
            xh8t = cpool.tile([128, KC, TQ * B], f8)
            nc.scalar.copy(out=xh8t[:], in_=xt[:])

            st = spool.tile([128, TQ, O_S], u8)

            for tt in range(TQ):
                tb = slice(tt * B, (tt + 1) * B)
                # corr first so its DVE combine overlaps the main matmuls
                ps_corr = psc.tile([128, O_S], f32, tag="c")
                for c in range(KC // 2):
                    nc.tensor.matmul(
                        ps_corr[:],
                        lhsT=xl[:, 2 * c : 2 * c + 2, tb],
                        rhs=w8sb[:, 2 * c : 2 * c + 2, :],
                        perf_mode=DR,
                        start=(c == 0),
                        stop=False,
                    )
                for c in range(KC // 2):
                    nc.tensor.matmul(
                        ps_corr[:],
                        lhsT=xh8t[:, 2 * c : 2 * c + 2, tb],
                        rhs=wl8sb[:, 2 * c : 2 * c + 2, :],
                        perf_mode=DR,
                        start=False,
                        stop=(c == KC // 2 - 1),
                    )
                ps_main = psm.tile([128, O_S], f32, tag="m")
                for k in range(KC):
                    nc.tensor.matmul(
                        ps_main[:],
                        lhsT=xt[:, k, tb],
                        rhs=wt16[:, k, :],
                        start=(k == 0),
                        stop=(k == KC - 1),
                    )
                # m = (corr * 2^-16 + w) + main
                nc.vector.scalar_tensor_tensor(
                    m_sb[:],
                    ps_corr[:],
                    CORR_SCALE,
                    w_sb[:],
                    mybir.AluOpType.mult,
                    mybir.AluOpType.add,
                )
                nc.vector.tensor_tensor(
                    m_sb[:], m_sb[:], ps_main[:], mybir.AluOpType.add
                )
                nc.vector.tensor_scalar(
                    st[:, tt, :], m_sb[:], 1.0, None, mybir.AluOpType.is_gt
                )
                sb = fpool.tile([128, O_S], f32)
                nc.vector.tensor_tensor(
                    sb[:], st[:, tt, :], bias_full[:], mybir.AluOpType.subtract
                )
                nc.vector.scalar_tensor_tensor(
                    w_sb[:],
                    m_sb[:],
                    BETA,
                    sb[:],
                    mybir.AluOpType.mult,
                    mybir.AluOpType.subtract,
                )
            nc.sync.dma_start(
                out=spk[:, tq * TQ : (tq + 1) * TQ, :], in_=st[:]
            )

    _slim_waits(nc)
    _split_waits(nc)
    return nc


def _build_nc_fp16(mode):
    from contextlib import ExitStack

    import concourse.bass as bass
    import concourse.mybir as mybir
    import concourse.tile as tile

    f32 = mybir.dt.float32
    split16 = mode == "fp16x2"
    if split16:
        dt_mm = mybir.dt.float16
    elif mode == "f32r":
        dt_mm = mybir.dt.float32r
    else:
        dt_mm = f32

    nc = bass.Bass()
    # xT planes: fp16x2 ships (hi, lo); other modes use plane 0 only
    n_planes = 2 if split16 else 1
    xT = nc.declare_dram_parameter("xT", [n_planes, NI, T * B], dt_mm, isOutput=False)
    WTs = nc.declare_dram_parameter("WTs", [n_planes, NI, O_S], dt_mm, isOutput=False)
    # cols 0..127: ones (lhsT of K=1 bias seed matmuls); then bias plane(s)
    ob = nc.declare_dram_parameter(
        "ob", [1, 128 + n_planes * O_S], dt_mm, isOutput=False
    )
    spk = nc.declare_dram_parameter("spk", [T, B, O_S], f32, isOutput=True)

    TQ = 4  # timesteps per DMA batch (>=512B contiguous runs, fewer DMAs)
    assert T % TQ == 0

    with tile.TileContext(nc) as tc, ExitStack() as ctx:
        singles = ctx.enter_context(tc.tile_pool(name="singles", bufs=1))
        xpool = ctx.enter_context(tc.tile_pool(name="xp", bufs=2))
        spool = ctx.enter_context(tc.tile_pool(name="sp", bufs=3))
        sbpool = ctx.enter_context(tc.tile_pool(name="sb", bufs=2))
        psum = ctx.enter_context(tc.tile_pool(name="ps", bufs=6, space="PSUM"))

        xTr = xT[:].rearrange("h (k p) tb -> p h k tb", p=128)

        # Prefetch the first timestep batch before the W preload so the
        # pass-1 matmuls start as early as possible.
        xt0 = xpool.tile([128, n_planes, KC, TQ * B], dt_mm)
        nc.sync.dma_start(out=xt0[:], in_=xTr[:, :, :, : TQ * B])

        # W^T resident in SBUF: [i%128, plane, i//128, o]. Load per-plane
        # (hi first) so pass-1 matmuls can start before the lo plane lands.
        wt_sb = singles.tile([128, n_planes, KC, O_S], dt_mm)
        WTr = WTs[:].rearrange("h (k p) o -> p h k o", p=128)
        for h in range(n_planes):
            nc.sync.dma_start(out=wt_sb[:, h], in_=WTr[:, h])

        ob_sb = singles.tile([1, 128 + n_planes * O_S], dt_mm)
        nc.sync.dma_start(out=ob_sb[:], in_=ob[:])

        m_sb = singles.tile([128, O_S], f32)  # membrane potential
        w_sb = singles.tile([128, O_S], f32)  # carry: beta*m + b - spk

        # One-time bias broadcast via a ones x bias seed matmul; bias_full
        # then feeds the per-step carry update and the initial carry (the
        # recurrence folds "+b" into w, saving 2 PE seed matmuls per step).
        bias_full = singles.tile([128, O_S], f32)
        ps_b = psum.tile([128, O_S], f32, tag="c")
        for h in range(n_planes):
            nc.tensor.matmul(
                ps_b[:],
                lhsT=ob_sb[:, :128],
                rhs=ob_sb[:, 128 + h * O_S : 128 + (h + 1) * O_S],
                start=(h == 0),
                stop=(h == n_planes - 1),
            )
        nc.vector.tensor_copy(bias_full[:], ps_b[:])
        nc.vector.tensor_copy(w_sb[:], bias_full[:])  # w_init = b

        spk_r = spk[:].rearrange("(tq tt) b o -> tq b tt o", tt=TQ)

        for tq in range(T // TQ):
            if tq == 0:
                xt = xt0
            else:
                xt = xpool.tile([128, n_planes, KC, TQ * B], dt_mm)
                nc.sync.dma_start(
                    out=xt[:], in_=xTr[:, :, :, tq * TQ * B : (tq + 1) * TQ * B]
                )
            st = spool.tile([128, TQ, O_S], f32)

            for tt in range(TQ):
                ps = psum.tile([128, O_S], f32, tag="c")
                if split16:
                    # xh*Wh, xh*Wl (shared stationary xh), then xl*Wh
                    passes = ((0, 0), (0, 1), (1, 0))
                else:
                    passes = ((0, 0),)
                mms = [(k, hx, hw) for k in range(KC) for hx, hw in passes]
                for i, (k, hx, hw) in enumerate(mms):
                    nc.tensor.matmul(
                        ps[:],
                        lhsT=xt[:, hx, k, tt * B : (tt + 1) * B],
                        rhs=wt_sb[:, hw, k, :],
                        start=(i == 0),
                        stop=(i == len(mms) - 1),
                    )
                nc.vector.tensor_tensor(m_sb[:], w_sb[:], ps[:], mybir.AluOpType.add)
                nc.vector.tensor_scalar(
                    st[:, tt, :], m_sb[:], 1.0, None, mybir.AluOpType.is_gt
                )
                sb = sbpool.tile([128, O_S], f32)
                nc.vector.tensor_tensor(
                    sb[:], st[:, tt, :], bias_full[:], mybir.AluOpType.subtract
                )
                nc.vector.scalar_tensor_tensor(
                    w_sb[:],
                    m_sb[:],
                    BETA,
                    sb[:],
                    mybir.AluOpType.mult,
                    mybir.AluOpType.subtract,
                )
            # one store per TQ steps: dst [b part, tt, o] view of spk[tq]
            nc.sync.dma_start(out=spk_r[tq], in_=st[:])

    _slim_waits(nc)
    _split_waits(nc)
    return nc


def _slim_waits(nc):
    """Drop sync waits already implied by earlier ones (transitive closure).

    Each engine queue dispatches in FIFO order, so a wait satisfied on an
    earlier instruction of the same queue covers later instructions. A wait
    on sem s >= v also imports everything the incrementing instruction's
    queue had itself waited for when it raised s to v (semaphore vector
    clocks with snapshots at each increment).
    """
    FRAMEWORK_OPS = ("InstEventSemaphore", "InstDrain")
    engine_clock = {}  # engine -> {sem_id: value known reached}
    totals = {}  # sem_id -> running total of increments
    snapshots = {}  # sem_id -> [(value, clock dict)] in increasing value order
    poisoned = set()  # sems touched by non-monotonic updates (barriers)

    def join(dst, src):
        for s, v in src.items():
            if s in poisoned:
                continue
            if dst.get(s, -1) < v:
                dst[s] = v

    for blk in nc.m.functions[0].blocks:
        for inst in blk.instructions:
            si = getattr(inst, "sync_info", None)
            if si is None:
                continue
            is_framework = type(inst).__name__ in FRAMEWORK_OPS
            clock = engine_clock.setdefault(inst.engine, {})
            if si.on_wait:
                kept = []
                for w in si.on_wait:
                    if (
                        w.sync_type != "semaphore"
                        or w.wait_mode != "sem-ge-imm"
                        or w.id in poisoned
                    ):
                        kept.append(w)
                        continue
                    covered = clock.get(w.id, -1) >= w.wait_value
                    for val, snap in snapshots.get(w.id, ()):
                        if val <= w.wait_value:
                            join(clock, snap)
                        else:
                            break
                    if clock.get(w.id, -1) < w.wait_value:
                        clock[w.id] = w.wait_value
                    if is_framework or not covered:
                        kept.append(w)
                si.on_wait = kept
            if si.on_update:
                for u in si.on_update:
                    if u.sync_type != "semaphore":
                        continue
                    if u.update_mode not in ("sem-inc", "sem-add-imm"):
                        # barrier-style sem: stop reasoning about it entirely
                        poisoned.add(u.id)
                        totals.pop(u.id, None)
                        snapshots.pop(u.id, None)
                        for c in engine_clock.values():
                            c.pop(u.id, None)
                        continue
                    if u.id in poisoned:
                        continue
                    tot = totals.get(u.id, 0) + (u.update_value or 1)
                    totals[u.id] = tot
                    snap = dict(clock)
                    snap[u.id] = tot
                    snapshots.setdefault(u.id, []).append((tot, snap))


def _split_waits(nc, limit=1):
    """Move excess sync waits onto injected same-queue NoOps.

    Walrus codegen accepts at most `limit` sync-wait commands per engine
    instruction on this target. Engine queues dispatch in order, so a
    preceding NoOp carrying the wait is equivalent.
    """
    import concourse.mybir as mybir

    n_nops = 0
    for blk in nc.m.functions[0].blocks:
        out = []
        changed = False
        for inst in blk.instructions:
            si = getattr(inst, "sync_info", None)
            if type(inst).__name__ == "InstEventSemaphore":
                out.append(inst)
                continue
            if si is not None and si.on_wait and len(si.on_wait) > limit:
                waits = list(si.on_wait)
                for w in waits[:-limit]:
                    nop = mybir.InstNoOp(name=f"wnop-{n_nops}", ins=[], outs=[])
                    n_nops += 1
                    nop.engine = inst.engine
                    nop.sync_info = mybir.SyncInfo(on_wait=[w], on_update=[])
                    nop.bass_nofuse = True
                    out.append(nop)
                    changed = True
                si.on_wait = waits[-limit:]
            out.append(inst)
        if changed:
            try:
                blk.instructions = out
            except Exception:
                blk.instructions.clear()
                blk.instructions.extend(out)


def _split16(a):
    hi = a.astype(np.float16)
    lo = (a - hi.astype(np.float32)).astype(np.float16)
    return hi, lo


def _prepare_in_maps(x, W, b):
    import ml_dtypes

    e4 = ml_dtypes.float8_e4m3

    x = np.ascontiguousarray(x, dtype=np.float32)
    W = np.ascontiguousarray(W, dtype=np.float32)
    b = np.ascontiguousarray(b, dtype=np.float32)
    # row tb = t*B + b so a 128-column block of xT = one full timestep
    x2 = x.reshape(T * B, NI)
    if MODE == "hybrid8":
        xh = x2.astype(np.float16)
        xl8 = ((x2 - xh.astype(np.float32)) * 2.0**11).astype(e4)
        xh16T = np.ascontiguousarray(xh.T)
        xl8T = np.ascontiguousarray(xl8.T)
        Wh = W.astype(np.float16)
        W8f = (W * 2.0**5).astype(e4)
        Wl8f = ((W - Wh.astype(np.float32)) * 2.0**16).astype(e4)
        Wh16T = np.ascontiguousarray(Wh.T)  # [NI, NO]
        W8T = np.ascontiguousarray(W8f.T)
        Wl8T = np.ascontiguousarray(Wl8f.T)
        bh, bl = _split16(b)
        in_maps = []
        for c in range(NCORES):
            sl = slice(c * O_S, (c + 1) * O_S)
            ob = np.empty((1, 128 + 2 * O_S), np.float16)
            ob[0, :128] = 1.0
            ob[0, 128 : 128 + O_S] = bh[sl]
            ob[0, 128 + O_S :] = bl[sl]
            in_maps.append(
                {
                    "xh16": xh16T,
                    "xl8": xl8T,
                    "Wh16": np.ascontiguousarray(Wh16T[:, sl]),
                    "W8": np.ascontiguousarray(W8T[:, sl]),
                    "Wl8": np.ascontiguousarray(Wl8T[:, sl]),
                    "ob": ob,
                }
            )
        return in_maps
    if MODE == "fp16x2":
        xh, xl = _split16(x2)
        xT = np.stack([np.ascontiguousarray(xh.T), np.ascontiguousarray(xl.T)])
        Wh, Wl = _split16(W)
        WTs_full = np.stack([np.ascontiguousarray(Wh.T), np.ascontiguousarray(Wl.T)])
        bh, bl = _split16(b)
        b_planes = [bh, bl]
        npdt = np.float16
    else:
        xT = np.ascontiguousarray(x2.T)[None]
        WTs_full = np.ascontiguousarray(W.T)[None]
        b_planes = [b]
        npdt = np.float32
    n_planes = len(b_planes)
    in_maps = []
    for c in range(NCORES):
        ob = np.empty((1, 128 + n_planes * O_S), npdt)
        ob[0, :128] = 1.0
        for h in range(n_planes):
            ob[0, 128 + h * O_S : 128 + (h + 1) * O_S] = b_planes[h][
                c * O_S : (c + 1) * O_S
            ]
        in_maps.append(
            {
                "xT": xT,
                "WTs": np.ascontiguousarray(WTs_full[:, :, c * O_S : (c + 1) * O_S]),
                "ob": ob,
            }
        )
    return in_maps


def run(x, W, b, trace=False):
    """Run the kernel; returns (out [T,B,NO] fp32, BassKernelResults)."""
    from concourse.bass_utils import run_bass_kernel_spmd

    if MODE not in _cache:
        _cache[MODE] = _build_nc(MODE)
    nc = _cache[MODE]
    in_maps = _prepare_in_maps(x, W, b)
    res = run_bass_kernel_spmd(nc, in_maps, list(range(NCORES)), trace=trace)
    if MODE == "hybrid8":
        # per-core spk is [B, T, O_S] uint8
        out = np.concatenate(
            [res.results[c]["spk"].transpose(1, 0, 2) for c in range(NCORES)],
            axis=2,
        ).astype(np.float32)
    else:
        out = np.concatenate([res.results[c]["spk"] for c in range(NCORES)], axis=2)
    return out, res


def kernel(x, W, b):
    out, _ = run(x, W, b, trace=False)
    return out
